# revision 56
# baseline (speedup 1.0000x reference)
"""Trainium2 Bass kernel for nn_AttentionModel (gnn_message_passing).

Distribution (8 cores):
  - Queries (M=8192) sharded into 8 contiguous chunks of 1024. idx is sorted,
    so each core's queries live in a contiguous window of sequences; the core
    receives h_grp (bf16) for just that window (row-major for gathers +
    transposed for matmuls).
  - segment_sum z: sharded by group. Each core computes z rows [512d, 512d+512)
    as a dense count-matrix matmul  z_d = C_d @ tok_emb  (both bf16; counts
    are tiny integers so C is exact), then AllGather (bf16, Shared output).
  - Attention is block-diagonal: queries of one sequence attend to its own 64
    positions. Blocks of BS=8 sequences; per-block query slots padded to a
    uniform CAP so the SPMD program is static.
  - q rows and z rows are fetched with batched dma_gather calls (<=1024
    indices each, the SWDGE descriptor-ring limit), spread across 4 SWDGE
    queues for parallel descriptor generation.
  - The (sequence-window + msk) mask is folded into the scores matmul as an
    extra K=16 rank-1 block (one-hot slot rows x per-column mask rows), so
    there is no amask tensor and no DVE mask add.
  - All compute-heavy matmuls run in bf16.
"""

import numpy as np

N_SEQ, L, DH, DX, M, G, N_TOK, N_MEM, N_TYP = 1024, 64, 256, 128, 8192, 4096, 10000, 262144, 64
NC = 8
MC = M // NC            # queries per core
GC = G // NC            # z-groups per core
NT_PAD = ((N_TOK + 511) // 512) * 512   # 10240
KT = NT_PAD // 128
KT4 = KT // 4           # 4-k-tile DMA batches
SCALE = 1.0 / np.sqrt(np.float32(DH))
NEG = -1.0e9
_SHARED_AG = True

_cache = {}


def _build(W, NBLK, BS, CAP, SLOT_PAD):
    import concourse.bacc as bacc
    import concourse.bass as bass
    import concourse.mybir as mybir
    import concourse.tile as tile
    from concourse.masks import make_identity
    from bass_rust import add_dep_helper

    f32 = mybir.dt.float32
    i16 = mybir.dt.int16
    bf16 = mybir.dt.bfloat16
    LB = BS * L                      # l-columns per block (512 for BS=8)
    NLT = LB // 128                  # l-chunks per block (4)
    NCH = SLOT_PAD // 512            # 512-slot chunks
    WL = W * L
    SB = 3                           # h superblock (NBLK % 3 == 0)

    f8 = mybir.dt.float8e4
    nc = bacc.Bacc("TRN2", target_bir_lowering=False, num_swdge_queues=4)

    hwin = nc.declare_dram_parameter("hwin", [WL, DH], bf16, isOutput=False)
    hwinT = nc.declare_dram_parameter("hwinT", [DH, WL], bf16, isOutput=False)
    tokh = nc.declare_dram_parameter("tokh", [128, KT, DX], bf16, isOutput=False)
    cmat = nc.declare_dram_parameter("cmat", [128, KT4, 4, GC], bf16, isOutput=False)
    wqT = nc.declare_dram_parameter("wqT", [DH, 2 * DH], f32, isOutput=False)
    wkT = nc.declare_dram_parameter("wkT", [DH, DH], f32, isOutput=False)
    bq = nc.declare_dram_parameter("bq", [128, 2], f32, isOutput=False)
    wrel = nc.declare_dram_parameter("wrel", [2 * DH + DX, N_TYP], f32, isOutput=False)
    brel = nc.declare_dram_parameter("brel", [N_TYP, 1], f32, isOutput=False)
    qgi = nc.declare_dram_parameter("qgi", [128, 2 * SLOT_PAD // 16], i16, isOutput=False)
    zgi = nc.declare_dram_parameter("zgi", [128, NBLK * LB // 16], i16, isOutput=False)
    onehot = nc.declare_dram_parameter("onehot", [16, SLOT_PAD], bf16, isOutput=False)
    winmask = nc.declare_dram_parameter("winmask", [16, WL], bf16, isOutput=False)
    logitT = nc.declare_dram_parameter("logitT", [N_TYP, SLOT_PAD], f32, isOutput=True)

    z_my = nc.dram_tensor("z_my", [GC, DX], bf16)
    z_all = nc.dram_tensor("z_all", [G, DX], bf16, addr_space="Shared" if _SHARED_AG else "Local")

    with tile.TileContext(nc) as tc:
        with (
            tc.tile_pool(name="const", bufs=1) as const,
            tc.tile_pool(name="persist", bufs=1) as persist,
            tc.tile_pool(name="zstream", bufs=2) as zstream,
            tc.tile_pool(name="blk", bufs=2) as blk,
            tc.tile_pool(name="soft", bufs=3) as soft,
        ):
            ident0 = const.tile([128, 128], f32)
            make_identity(nc, ident0[:])
            # DVE-homed identities: PE transposes then depend on one engine sem.
            ident = const.tile([128, 128], f32, tag="identW")
            nc.vector.tensor_copy(ident[:], ident0[:])
            ident_bf = const.tile([128, 128], bf16, tag="identB")
            nc.vector.tensor_copy(ident_bf[:], ident0[:])

            # ---- weights / small inputs ----
            # token table first: the z matmul chain needs it earliest
            th_all = persist.tile([128, KT, DX], bf16, tag="tokh")
            nc.scalar.dma_start(th_all[:, :KT // 2, :], tokh[:, :KT // 2, :])
            nc.scalar.dma_start(th_all[:, KT // 2:, :], tokh[:, KT // 2:, :])
            wqT_sb = [persist.tile([128, 2 * DH], f32, tag=f"wqT{j}", name=f"wqT{j}") for j in range(2)]
            for j in range(2):
                nc.scalar.dma_start(wqT_sb[j][:], wqT[j * 128:(j + 1) * 128, :])
            wkT_sb = [persist.tile([128, DH], f32, tag=f"wkT{j}", name=f"wkT{j}") for j in range(2)]
            for j in range(2):
                nc.scalar.dma_start(wkT_sb[j][:], wkT[j * 128:(j + 1) * 128, :])
            bq_sb = persist.tile([128, 2], f32, tag="bq")
            nc.scalar.dma_start(bq_sb[:], bq[:])
            wrel_sb = [persist.tile([128, N_TYP], bf16, tag=f"wrel{k}", name=f"wrel{k}") for k in range(5)]
            wrel_f32 = [persist.tile([128, N_TYP], f32, tag=f"wrelF{k}", name=f"wrelF{k}") for k in range(5)]
            for k in range(5):
                nc.scalar.dma_start(wrel_f32[k][:], wrel[k * 128:(k + 1) * 128, :])
                nc.vector.tensor_copy(wrel_sb[k][:], wrel_f32[k][:])
            brel_sb = persist.tile([N_TYP, 1], f32, tag="brel")
            nc.scalar.dma_start(brel_sb[:], brel[:])
            qgi_sb = persist.tile([128, 2 * SLOT_PAD // 16], i16, tag="qgi")
            nc.scalar.dma_start(qgi_sb[:], qgi[:])
            zgi_sb = persist.tile([128, NBLK * LB // 16], i16, tag="zgi")
            nc.scalar.dma_start(zgi_sb[:], zgi[:])
            onehot_sb = persist.tile([16, SLOT_PAD], bf16, tag="onehot")
            nc.scalar.dma_start(onehot_sb[:], onehot[:])
            winmask_sb = persist.tile([16, WL], bf16, tag="winmask")
            nc.scalar.dma_start(winmask_sb[:], winmask[:])

            wqk_sb = [persist.tile([128, DH], bf16, tag=f"wqk{a}", name=f"wqk{a}") for a in range(4)]
            bqk_sb = [persist.tile([128, 1], f32, tag=f"bqk{c}", name=f"bqk{c}") for c in range(2)]

            # front PSUM pools: 1 + 1 + 2 + 2 + 2 = 8 banks
            zps_cm = tc.tile_pool(name="zps", bufs=1, space="PSUM"); zps = zps_cm.__enter__()
            ztps_cm = tc.tile_pool(name="ztps", bufs=1, space="PSUM"); ztps = ztps_cm.__enter__()
            qtps_cm = tc.tile_pool(name="qtps", bufs=2, space="PSUM"); qtps = qtps_cm.__enter__()
            qkps_cm = tc.tile_pool(name="qkps", bufs=2, space="PSUM"); qkps = qkps_cm.__enter__()
            lqps_cm = tc.tile_pool(name="lqps", bufs=2, space="PSUM"); lqps = lqps_cm.__enter__()

            # ---- phase Z: z_d^T = tok^T @ C_d (bf16; counts shipped fp8,
            # upconverted on DVE/ACT), transpose, AllGather ----
            zdT = persist.tile([DX, GC], f32, tag="zdT")
            zrow = persist.tile([128, GC // 128 * DX], bf16, tag="zrow")
            zpsum = zps.tile([DX, GC], f32)
            # SWDGE sem lanes (8, round-robin over Pool DMA instrs) are locked
            # to one queue each: queue = lane %% 4 keeps the lane->queue map
            # consistent while every 4-instruction wave spans all 4 queues.
            pool_dma_n = [0]

            def pool_q():
                q = pool_dma_n[0] % 4
                pool_dma_n[0] += 1
                return q

            # PE warmup: ~4us of dependency-free transposes so the HAM clock
            # gate opens (1.2 -> 2.4 GHz) before the z matmul chain starts.
            wrm = ztps.tile([128, GC // 128, 128], f32, tag="ztp")
            for i in range(36):
                nc.tensor.transpose(wrm[:, i % (GC // 128), :], ident[:], ident[:])

            c8_early = None
            for cb in range(KT4 // 2):      # 2 kb per HWDGE DMA chunk
                ck = zstream.tile([128, 2, 4, GC], bf16, tag="ck", bufs=4)
                c8_dma = nc.sync.dma_start(ck[:], cmat[:, 2 * cb:2 * cb + 2])
                if cb == 2:
                    c8_early = c8_dma
                for h in range(2):
                    kb = 2 * cb + h
                    for j in range(4):
                        nc.tensor.matmul(zpsum[:], lhsT=th_all[:, kb * 4 + j, :],
                                         rhs=ck[:, h, j, :],
                                         start=(kb == 0 and j == 0),
                                         stop=(kb == KT4 - 1 and j == 3))

            # ---- q gather: plain dma_gather (row-major), PE transposes ----
            # qg[p, j, :] = hwin[qidx[j*128+p], :]; j<16: src half, j>=16: dst.
            # SWDGE descriptor ring fits <=1024 idx per call. Deferred past the
            # z stream (dep on last cmat DMA) so the scheduler keeps all phase-Z
            # work ahead of gather-dependent PE instructions.
            qg = persist.tile([128, 2 * SLOT_PAD // 128, DH], bf16, tag="qg")
            for k in range(2 * SLOT_PAD // 1024):
                qg_inst = nc.gpsimd.dma_gather(
                    qg[:, k * 8:(k + 1) * 8, :], hwin[:],
                    qgi_sb[:, k * 64:(k + 1) * 64], 1024, 1024, DH,
                    queue_num=pool_q(), single_packet=False,
                )
                add_dep_helper(qg_inst.ins, c8_early.ins,
                               reason="stagger q gathers into the z stream")

            nc.vector.tensor_copy(zdT[:], zpsum[:])
            ptz = ztps.tile([128, GC // 128, 128], f32, tag="ztp")
            for c in range(GC // 128):
                nc.tensor.transpose(ptz[:, c, :], zdT[:, c * 128:(c + 1) * 128], ident[:])
            zrow_copy = nc.vector.tensor_copy(zrow[:], ptz[:])
            zmy_last = nc.sync.dma_start(
                z_my.rearrange("(c p) x -> p c x", p=128), zrow[:])
            ag_inst = nc.gpsimd.collective_compute(
                "AllGather", mybir.AluOpType.bypass,
                replica_groups=[list(range(NC))],
                ins=[z_my.ap().opt()], outs=[z_all.ap().opt()],
            )

            # ---- z gathers: dma_gather batches of 1024 rows (after AG) ----
            zg_all = persist.tile([128, NBLK * NLT, DX], bf16, tag="zg_all")
            for k in range(NBLK * LB // 1024):
                zg_inst = nc.gpsimd.dma_gather(
                    zg_all[:, k * 8:(k + 1) * 8, :], z_all[:],
                    zgi_sb[:, k * 64:(k + 1) * 64], 1024, 1024, DX,
                    queue_num=pool_q(), single_packet=False,
                )
                add_dep_helper(zg_inst.ins, ag_inst.ins,
                               reason="gather z after AllGather")

            # Wqk = Wq @ Wk^T (bf16); bqk = Wk @ bq  (after the z chain on PE)
            for a in range(4):
                pwt = qkps.tile([128, 512], f32, tag="qkp")
                for b in range(2):
                    nc.tensor.matmul(pwt[:, :DH], lhsT=wqT_sb[b][:, a * 128:(a + 1) * 128],
                                     rhs=wkT_sb[b][:], start=(b == 0), stop=(b == 1))
                nc.vector.tensor_copy(wqk_sb[a][:], pwt[:, :DH])
            for c in range(2):
                pb = qkps.tile([128, 512], f32, tag="qkp")
                for b in range(2):
                    nc.tensor.matmul(pb[:, :1], lhsT=wkT_sb[b][:, c * 128:(c + 1) * 128],
                                     rhs=bq_sb[:, b:b + 1],
                                     start=(b == 0), stop=(b == 1))
                nc.vector.tensor_copy(bqk_sb[c][:], pb[:, :1])

            # ---- phase QT/QK/LQ: transpose q, then qkT / logit_q ----
            qkT = [persist.tile([128, SLOT_PAD], bf16, tag=f"qkT{c}", name=f"qkT{c}") for c in range(2)]
            logit_q = persist.tile([N_TYP, SLOT_PAD], f32, tag="logit_q")
            with tc.tile_pool(name="qtm", bufs=2) as qtm:
                for sl in range(NCH):
                    qtmix = [None, None]
                    for half in range(2):
                        qm = qtm.tile([128, 4, 2, 128], bf16, tag=f"qTm{half}",
                                      name=f"qTm{half}")
                        qtmix[half] = qm
                        for t in range(4):
                            j = half * (SLOT_PAD // 128) + sl * 4 + t
                            pt = qtps.tile([128, 2, 128], bf16, tag="qtp")
                            for c in range(2):
                                tr = nc.tensor.transpose(pt[:, c, :],
                                                         qg[:, j, c * 128:(c + 1) * 128],
                                                         ident_bf[:])
                                if sl == 0 and half == 0 and t == 0 and c == 0:
                                    add_dep_helper(tr.ins, zmy_last.ins,
                                                   reason="keep z chain ahead of QT on PE")
                            qmc = nc.vector.tensor_copy(qm[:, t, :, :], pt[:])
                            if sl == 0 and half == 0 and t == 0:
                                add_dep_helper(qmc.ins, zrow_copy.ins,
                                               reason="keep z evac ahead of QT on DVE")

                    def qT_a(a):
                        return qtmix[a // 2][:, :, a % 2, :]

                    for c in range(2):
                        pq = qkps.tile([128, 512], f32, tag="qkp")
                        for a in range(4):
                            nc.tensor.matmul(pq[:], lhsT=wqk_sb[a][:, c * 128:(c + 1) * 128],
                                             rhs=qT_a(a), start=(a == 0), stop=(a == 3))
                        nc.scalar.activation(qkT[c][:, sl * 512:(sl + 1) * 512], pq[:],
                                             mybir.ActivationFunctionType.Identity,
                                             bias=bqk_sb[c][:, :1])
                    pl = lqps.tile([N_TYP, 512], f32, tag="lqp")
                    for a in range(4):
                        nc.tensor.matmul(pl[:], lhsT=wrel_sb[a][:], rhs=qT_a(a),
                                         start=(a == 0), stop=(a == 3))
                    nc.scalar.activation(logit_q[:, sl * 512:(sl + 1) * 512], pl[:],
                                         mybir.ActivationFunctionType.Identity,
                                         bias=brel_sb[:, :1])

            lqps_cm.__exit__(None, None, None)
            qkps_cm.__exit__(None, None, None)
            qtps_cm.__exit__(None, None, None)
            ztps_cm.__exit__(None, None, None)
            zps_cm.__exit__(None, None, None)

            # ---- phase S1: scores (with fused mask) / softmax / attnT ----
            aT = persist.tile([128, NBLK, NLT, CAP], bf16, tag="aT")
            hTb = [None, None]
            with (
                tc.tile_pool(name="sps", bufs=2, space="PSUM") as sps,
                tc.tile_pool(name="atps", bufs=3, space="PSUM") as atps,
            ):
                for b in range(NBLK):
                    if b % SB == 0:
                        for c in range(2):
                            hTb[c] = blk.tile([128, SB * LB], bf16, tag=f"hT{c}", name=f"hT{c}")
                            nc.scalar.dma_start(
                                hTb[c][:],
                                hwinT[c * 128:(c + 1) * 128, b * LB:(b + SB) * LB])
                    off = (b % SB) * LB

                    ps_s = sps.tile([CAP, LB], f32, tag="sps")
                    for c in range(2):
                        nc.tensor.matmul(ps_s[:], lhsT=qkT[c][:, b * CAP:b * CAP + CAP],
                                         rhs=hTb[c][:, off:off + LB],
                                         start=(c == 0), stop=False)
                    nc.tensor.matmul(ps_s[:], lhsT=onehot_sb[:, b * CAP:b * CAP + CAP],
                                     rhs=winmask_sb[:, b * LB:(b + 1) * LB],
                                     start=False, stop=True)
                    e = soft.tile([CAP, LB], bf16, tag="e", bufs=2)
                    den = soft.tile([CAP, 1], f32, tag="den")
                    nc.scalar.activation(e[:], ps_s[:], mybir.ActivationFunctionType.Exp,
                                         scale=float(SCALE), accum_out=den[:])
                    rec = soft.tile([CAP, 1], f32, tag="rec")
                    nc.vector.reciprocal(rec[:], den[:])
                    attn = soft.tile([CAP, LB], bf16, tag="attn", bufs=2)
                    nc.vector.tensor_scalar_mul(attn[:], e[:], rec[:])

                    pta = atps.tile([128, NLT, CAP], bf16, tag="atp")
                    for k in range(NLT):
                        nc.tensor.transpose(pta[:, k, :], attn[:, k * 128:(k + 1) * 128],
                                            ident_bf[:CAP, :CAP])
                    nc.vector.tensor_copy(aT[:, b, :, :], pta[:])

            # ---- phase S2: ctxT = zg^T @ aT ----
            ctxT = persist.tile([128, SLOT_PAD], bf16, tag="ctxT")
            if NBLK * CAP < SLOT_PAD:
                nc.vector.memset(ctxT[:, NBLK * CAP:], 0.0)
            with tc.tile_pool(name="cps", bufs=3, space="PSUM") as cps:
                for b in range(NBLK):
                    ps_c = cps.tile([DX, CAP], f32, tag="cps")
                    for k in range(NLT):
                        nc.tensor.matmul(ps_c[:], lhsT=zg_all[:, b * NLT + k, :],
                                         rhs=aT[:, b, k, :],
                                         start=(k == 0), stop=(k == NLT - 1))
                    nc.scalar.activation(ctxT[:, b * CAP:b * CAP + CAP], ps_c[:],
                                         mybir.ActivationFunctionType.Copy)

            # ---- phase L: logitT = logit_q + WrelC^T @ ctxT ----
            with tc.tile_pool(name="lps", bufs=2, space="PSUM") as lps:
                for ch in range(NCH):
                    pl = lps.tile([N_TYP, 512], f32, tag="lps")
                    nc.tensor.matmul(pl[:], lhsT=wrel_sb[4][:],
                                     rhs=ctxT[:, ch * 512:(ch + 1) * 512],
                                     start=True, stop=True)
                    lg = soft.tile([N_TYP, 512], f32, tag="lg", bufs=2)
                    nc.vector.tensor_add(lg[:], pl[:], logit_q[:, ch * 512:(ch + 1) * 512])
                    nc.sync.dma_start(logitT[:, ch * 512:(ch + 1) * 512], lg[:])

    nc.compile()
    return nc


def _wrap16(flat):
    """int16 gather-index layout: index i at [i % 16, i // 16], rows tiled to 128."""
    a = np.asarray(flat, np.int16).reshape(-1, 16).T
    return np.ascontiguousarray(np.tile(a, (8, 1)))


def _prep(mem, grp, pos2grp, h_grp, msk, idx, src, dst, typ, tok_emb, Wq, bq, Wk, bk, Wrel, brel):
    """Host-side sharding/layout. Integer index work + relayout only."""
    import ml_dtypes
    bf = ml_dtypes.bfloat16
    idx = np.asarray(idx, np.int64)
    src = np.asarray(src, np.int64)
    dst = np.asarray(dst, np.int64)
    mem = np.asarray(mem, np.int64)
    grp = np.asarray(grp, np.int64)
    pos2grp = np.asarray(pos2grp, np.int64)
    msk = np.asarray(msk)
    h_grp = np.asarray(h_grp, np.float32)
    tok_emb = np.asarray(tok_emb, np.float32)

    # ---- count matrix for segment_sum ----
    C = np.bincount(grp * N_TOK + mem, minlength=G * N_TOK).reshape(G, N_TOK).astype(np.float32)

    # ---- per-core windows ----
    starts = np.array([idx[d * MC] for d in range(NC)])
    ends = np.array([idx[(d + 1) * MC - 1] for d in range(NC)])
    BS = 8
    Wmax = int((ends - starts).max()) + 1
    W = -(-Wmax // (3 * BS)) * (3 * BS)

    maxc = 0
    for d in range(NC):
        blkid = (idx[d * MC:(d + 1) * MC] - starts[d]) // BS
        maxc = max(maxc, int(np.bincount(blkid).max()))
    if maxc > 128:
        BS = 4
        W = -(-Wmax // (3 * BS)) * (3 * BS)
        maxc = 0
        for d in range(NC):
            blkid = (idx[d * MC:(d + 1) * MC] - starts[d]) // BS
            maxc = max(maxc, int(np.bincount(blkid).max()))
        assert maxc <= 128, f"block occupancy {maxc} > 128 even at BS=4"
    CAP = -(-maxc // 32) * 32
    NBLK = W // BS
    SLOT_PAD = -(-(NBLK * CAP) // 1024) * 1024
    LB = BS * L

    tok_pad = np.vstack([tok_emb, np.zeros((NT_PAD - N_TOK, DX), np.float32)])
    # [128, KT, DX]: partition p holds rows t*128+p, contiguous per partition
    tok_bf = np.ascontiguousarray(
        tok_pad.astype(bf).reshape(KT, 128, DX).transpose(1, 0, 2))
    f8 = ml_dtypes.float8_e4m3
    wqT_h = np.ascontiguousarray(np.asarray(Wq, np.float32).T)
    wkT_h = np.ascontiguousarray(np.asarray(Wk, np.float32).T)
    bq_h = np.ascontiguousarray(np.asarray(bq, np.float32).reshape(2, 128).T)
    wrel_h = np.ascontiguousarray(np.asarray(Wrel, np.float32))
    brel_h = np.asarray(brel, np.float32).reshape(N_TYP, 1)

    h_flat = np.ascontiguousarray(h_grp.reshape(N_SEQ * L, DH))
    per_core = []
    slot_maps = []
    for d in range(NC):
        n_lo = int(starts[d])
        qid = idx[d * MC:(d + 1) * MC]
        qsrc = src[d * MC:(d + 1) * MC]
        qdst = dst[d * MC:(d + 1) * MC]

        hw = np.zeros((W * L, DH), np.float32)
        n_hi = min(n_lo + W, N_SEQ)
        hw[: (n_hi - n_lo) * L] = h_flat[n_lo * L: n_hi * L]
        hw_bf = hw.astype(bf)
        hwT_bf = np.ascontiguousarray(hw_bf.T)

        blkid = (qid - n_lo) // BS
        cnt = np.zeros(NBLK, np.int64)
        slot = np.zeros(MC, np.int64)
        for i in range(MC):
            b = blkid[i]
            slot[i] = b * CAP + cnt[b]
            cnt[b] += 1
        slot_maps.append(slot)

        qsi_h = np.zeros(SLOT_PAD, np.int64)
        qdi_h = np.zeros(SLOT_PAD, np.int64)
        qsi_h[slot] = (qid - n_lo) * L + qsrc
        qdi_h[slot] = (qid - n_lo) * L + qdst

        p2g_pad = np.zeros((W, L), np.int64)
        p2g_pad[: n_hi - n_lo] = pos2grp[n_lo:n_hi]

        # one-hot slot rows (which seq-in-block each slot belongs to)
        onehot_h = np.zeros((16, SLOT_PAD), np.float32)
        o = (qid - n_lo) % BS
        onehot_h[o, slot] = 1.0
        # per-column window masks: row j valid for cols of seq j-in-block
        winmask_h = np.full((16, W * L), 0.0, np.float32)
        wm = np.full((W, L), NEG, np.float32)
        valid = msk[n_lo:n_hi].astype(bool)
        wm[: n_hi - n_lo][valid] = 0.0
        wcol = np.arange(W) % BS
        for j in range(BS):
            rowj = np.full((W, L), NEG, np.float32)
            rowj[wcol == j] = wm[wcol == j]
            winmask_h[j] = rowj.reshape(-1)

        cm = np.vstack([C[d * GC:(d + 1) * GC].T,
                        np.zeros((NT_PAD - N_TOK, GC), np.float32)])
        per_core.append({
            "hwin": hw_bf, "hwinT": hwT_bf, "tokh": tok_bf,
            "cmat": np.ascontiguousarray(
                cm.astype(bf).reshape(KT4, 4, 128, GC).transpose(2, 0, 1, 3)),
            "wqT": wqT_h, "wkT": wkT_h, "bq": bq_h, "wrel": wrel_h, "brel": brel_h,
            "qgi": _wrap16(np.concatenate([qsi_h, qdi_h])),
            "zgi": _wrap16(p2g_pad.reshape(-1)),
            "onehot": onehot_h.astype(bf),
            "winmask": winmask_h.astype(bf),
        })
    return per_core, slot_maps, (W, NBLK, BS, CAP, SLOT_PAD)


def kernel(**inputs) -> np.ndarray:
    from concourse.bass_utils import run_bass_kernel_spmd

    per_core, slot_maps, key = _prep(**{k: inputs[k] for k in (
        "mem", "grp", "pos2grp", "h_grp", "msk", "idx", "src", "dst", "typ",
        "tok_emb", "Wq", "bq", "Wk", "bk", "Wrel", "brel")})
    if key not in _cache:
        _cache[key] = _build(*key)
    nc = _cache[key]
    res = run_bass_kernel_spmd(nc, per_core, core_ids=list(range(NC)))
    globals()["LAST_RESULT"] = res
    globals()["LAST_EXEC_NS"] = res.exec_time_ns
    out = np.empty((M, N_TYP), np.float32)
    for d in range(NC):
        out[d * MC:(d + 1) * MC] = res.results[d]["logitT"][:, slot_maps[d]].T
    return out


# revision 57
# speedup vs baseline: 1.0819x; 1.0819x over previous
"""Trainium2 Bass kernel for nn_AttentionModel (gnn_message_passing).

Distribution (8 cores):
  - Queries (M=8192) sharded into 8 contiguous chunks of 1024. idx is sorted,
    so each core's queries live in a contiguous window of sequences; the core
    receives h_grp (bf16) for just that window (row-major for gathers +
    transposed for matmuls).
  - segment_sum z: sharded by group. Each core computes z rows [512d, 512d+512)
    as a dense count-matrix matmul  z_d = C_d @ tok_emb  (both bf16; counts
    are tiny integers so C is exact), then AllGather (bf16, Shared output).
  - Attention is block-diagonal: queries of one sequence attend to its own 64
    positions. Blocks of BS=8 sequences; per-block query slots padded to a
    uniform CAP so the SPMD program is static.
  - q rows and z rows are fetched with batched dma_gather calls (<=1024
    indices each, the SWDGE descriptor-ring limit), spread across 4 SWDGE
    queues for parallel descriptor generation.
  - The (sequence-window + msk) mask is folded into the scores matmul as an
    extra K=16 rank-1 block (one-hot slot rows x per-column mask rows), so
    there is no amask tensor and no DVE mask add.
  - All compute-heavy matmuls run in bf16.
"""

import numpy as np

N_SEQ, L, DH, DX, M, G, N_TOK, N_MEM, N_TYP = 1024, 64, 256, 128, 8192, 4096, 10000, 262144, 64
NC = 8
MC = M // NC            # queries per core
GC = G // NC            # z-groups per core
NT_PAD = ((N_TOK + 511) // 512) * 512   # 10240
KT = NT_PAD // 128
KT4 = KT // 4           # 4-k-tile DMA batches
SCALE = 1.0 / np.sqrt(np.float32(DH))
NEG = -1.0e9
_SHARED_AG = True

_cache = {}


def _build(W, NBLK, BS, CAP, SLOT_PAD):
    import concourse.bacc as bacc
    import concourse.bass as bass
    import concourse.mybir as mybir
    import concourse.tile as tile
    from concourse.masks import make_identity
    from bass_rust import add_dep_helper

    f32 = mybir.dt.float32
    i16 = mybir.dt.int16
    bf16 = mybir.dt.bfloat16
    LB = BS * L                      # l-columns per block (512 for BS=8)
    NLT = LB // 128                  # l-chunks per block (4)
    NCH = SLOT_PAD // 512            # 512-slot chunks
    WL = W * L
    SB = 3                           # h superblock (NBLK % 3 == 0)

    f8 = mybir.dt.float8e4
    nc = bacc.Bacc("TRN2", target_bir_lowering=False, num_swdge_queues=4)

    hwin = nc.declare_dram_parameter("hwin", [WL, DH], bf16, isOutput=False)
    hwinT = nc.declare_dram_parameter("hwinT", [DH, WL], bf16, isOutput=False)
    tokh = nc.declare_dram_parameter("tokh", [128, KT, DX], bf16, isOutput=False)
    cmat = nc.declare_dram_parameter("cmat", [128, KT4, 4, GC], bf16, isOutput=False)
    wqT = nc.declare_dram_parameter("wqT", [DH, 2 * DH], f32, isOutput=False)
    wkT = nc.declare_dram_parameter("wkT", [DH, DH], f32, isOutput=False)
    bq = nc.declare_dram_parameter("bq", [128, 2], f32, isOutput=False)
    wrel = nc.declare_dram_parameter("wrel", [2 * DH + DX, N_TYP], f32, isOutput=False)
    brel = nc.declare_dram_parameter("brel", [N_TYP, 1], f32, isOutput=False)
    qgi = nc.declare_dram_parameter("qgi", [128, 2 * SLOT_PAD // 16], i16, isOutput=False)
    zgi = nc.declare_dram_parameter("zgi", [128, NBLK * LB // 16], i16, isOutput=False)
    onehot = nc.declare_dram_parameter("onehot", [16, SLOT_PAD], bf16, isOutput=False)
    winmask = nc.declare_dram_parameter("winmask", [16, WL], bf16, isOutput=False)
    logitT = nc.declare_dram_parameter("logitT", [N_TYP, SLOT_PAD], f32, isOutput=True)

    z_my = nc.dram_tensor("z_my", [GC, DX], bf16)
    z_all = nc.dram_tensor("z_all", [G, DX], bf16, addr_space="Shared" if _SHARED_AG else "Local")

    with tile.TileContext(nc) as tc:
        with (
            tc.tile_pool(name="const", bufs=1) as const,
            tc.tile_pool(name="persist", bufs=1) as persist,
            tc.tile_pool(name="zstream", bufs=2) as zstream,
            tc.tile_pool(name="blk", bufs=2) as blk,
            tc.tile_pool(name="soft", bufs=3) as soft,
        ):
            ident0 = const.tile([128, 128], f32)
            make_identity(nc, ident0[:])
            # DVE-homed identities: PE transposes then depend on one engine sem.
            ident = const.tile([128, 128], f32, tag="identW")
            nc.vector.tensor_copy(ident[:], ident0[:])
            ident_bf = const.tile([128, 128], bf16, tag="identB")
            nc.vector.tensor_copy(ident_bf[:], ident0[:])

            # ---- weights / small inputs ----
            # token table first: the z matmul chain needs it earliest
            th_all = persist.tile([128, KT, DX], bf16, tag="tokh")
            nc.scalar.dma_start(th_all[:, :KT // 2, :], tokh[:, :KT // 2, :])
            nc.scalar.dma_start(th_all[:, KT // 2:, :], tokh[:, KT // 2:, :])
            wqT_sb = [persist.tile([128, 2 * DH], f32, tag=f"wqT{j}", name=f"wqT{j}") for j in range(2)]
            for j in range(2):
                nc.scalar.dma_start(wqT_sb[j][:], wqT[j * 128:(j + 1) * 128, :])
            wkT_sb = [persist.tile([128, DH], f32, tag=f"wkT{j}", name=f"wkT{j}") for j in range(2)]
            for j in range(2):
                nc.scalar.dma_start(wkT_sb[j][:], wkT[j * 128:(j + 1) * 128, :])
            bq_sb = persist.tile([128, 2], f32, tag="bq")
            nc.scalar.dma_start(bq_sb[:], bq[:])
            wrel_sb = [persist.tile([128, N_TYP], bf16, tag=f"wrel{k}", name=f"wrel{k}") for k in range(5)]
            wrel_f32 = [persist.tile([128, N_TYP], f32, tag=f"wrelF{k}", name=f"wrelF{k}") for k in range(5)]
            for k in range(5):
                nc.scalar.dma_start(wrel_f32[k][:], wrel[k * 128:(k + 1) * 128, :])
                nc.vector.tensor_copy(wrel_sb[k][:], wrel_f32[k][:])
            brel_sb = persist.tile([N_TYP, 1], f32, tag="brel")
            nc.scalar.dma_start(brel_sb[:], brel[:])
            qgi_sb = persist.tile([128, 2 * SLOT_PAD // 16], i16, tag="qgi")
            nc.scalar.dma_start(qgi_sb[:], qgi[:])
            zgi_sb = persist.tile([128, NBLK * LB // 16], i16, tag="zgi")
            nc.scalar.dma_start(zgi_sb[:], zgi[:])
            onehot_sb = persist.tile([16, SLOT_PAD], bf16, tag="onehot")
            nc.scalar.dma_start(onehot_sb[:], onehot[:])
            winmask_sb = persist.tile([16, WL], bf16, tag="winmask")
            nc.scalar.dma_start(winmask_sb[:], winmask[:])

            wqk_sb = [persist.tile([128, DH], bf16, tag=f"wqk{a}", name=f"wqk{a}") for a in range(4)]
            bqk_sb = [persist.tile([128, 1], f32, tag=f"bqk{c}", name=f"bqk{c}") for c in range(2)]

            # front PSUM pools: 1 + 1 + 2 + 2 + 2 = 8 banks
            zps_cm = tc.tile_pool(name="zps", bufs=1, space="PSUM"); zps = zps_cm.__enter__()
            ztps_cm = tc.tile_pool(name="ztps", bufs=1, space="PSUM"); ztps = ztps_cm.__enter__()
            qtps_cm = tc.tile_pool(name="qtps", bufs=2, space="PSUM"); qtps = qtps_cm.__enter__()
            qkps_cm = tc.tile_pool(name="qkps", bufs=2, space="PSUM"); qkps = qkps_cm.__enter__()
            lqps_cm = tc.tile_pool(name="lqps", bufs=2, space="PSUM"); lqps = lqps_cm.__enter__()

            # ---- phase Z: z_d^T = tok^T @ C_d (bf16; counts shipped fp8,
            # upconverted on DVE/ACT), transpose, AllGather ----
            zdT = persist.tile([DX, GC], f32, tag="zdT")
            zrow = persist.tile([128, GC // 128 * DX], bf16, tag="zrow")
            zpsum = zps.tile([DX, GC], f32)
            # SWDGE sem lanes (8, round-robin over Pool DMA instrs) are locked
            # to one queue each: queue = lane %% 4 keeps the lane->queue map
            # consistent while every 4-instruction wave spans all 4 queues.
            pool_dma_n = [0]

            def pool_q():
                q = pool_dma_n[0] % 4
                pool_dma_n[0] += 1
                return q

            # PE warmup: ~4us of dependency-free transposes so the HAM clock
            # gate opens (1.2 -> 2.4 GHz) before the z matmul chain starts.
            wrm = ztps.tile([128, GC // 128, 128], f32, tag="ztp")
            for i in range(36):
                nc.tensor.transpose(wrm[:, i % (GC // 128), :], ident[:], ident[:])

            c8_early = None
            for cb in range(KT4 // 2):      # 2 kb per HWDGE DMA chunk
                ck = zstream.tile([128, 2, 4, GC], bf16, tag="ck", bufs=4)
                c8_dma = nc.sync.dma_start(ck[:], cmat[:, 2 * cb:2 * cb + 2])
                if cb == 2:
                    c8_early = c8_dma
                for h in range(2):
                    kb = 2 * cb + h
                    for j in range(4):
                        nc.tensor.matmul(zpsum[:], lhsT=th_all[:, kb * 4 + j, :],
                                         rhs=ck[:, h, j, :],
                                         start=(kb == 0 and j == 0),
                                         stop=(kb == KT4 - 1 and j == 3))

            # ---- q gather: plain dma_gather (row-major), PE transposes ----
            # qg[p, j, :] = hwin[qidx[j*128+p], :]; j<16: src half, j>=16: dst.
            # SWDGE descriptor ring fits <=1024 idx per call. Deferred past the
            # z stream (dep on last cmat DMA) so the scheduler keeps all phase-Z
            # work ahead of gather-dependent PE instructions.
            qg = persist.tile([128, 2 * SLOT_PAD // 128, DH], bf16, tag="qg")
            for k in range(2 * SLOT_PAD // 1024):
                qg_inst = nc.gpsimd.dma_gather(
                    qg[:, k * 8:(k + 1) * 8, :], hwin[:],
                    qgi_sb[:, k * 64:(k + 1) * 64], 1024, 1024, DH,
                    queue_num=pool_q(), single_packet=False,
                )
                add_dep_helper(qg_inst.ins, c8_early.ins,
                               reason="stagger q gathers into the z stream")

            nc.vector.tensor_copy(zdT[:], zpsum[:])
            ptz = ztps.tile([128, GC // 128, 128], f32, tag="ztp")
            for c in range(GC // 128):
                nc.tensor.transpose(ptz[:, c, :], zdT[:, c * 128:(c + 1) * 128], ident[:])
            zrow_copy = nc.vector.tensor_copy(zrow[:], ptz[:])
            zmy_last = nc.gpsimd.dma_start(
                z_my.rearrange("(c p) x -> p c x", p=128), zrow[:])
            pool_q()  # account the SWDGE lane used by the z_my write
            ag_inst = nc.gpsimd.collective_compute(
                "AllGather", mybir.AluOpType.bypass,
                replica_groups=[list(range(NC))],
                ins=[z_my.ap().opt()], outs=[z_all.ap().opt()],
            )

            # ---- z gathers: dma_gather batches of 1024 rows (after AG) ----
            zg_all = persist.tile([128, NBLK * NLT, DX], bf16, tag="zg_all")
            for k in range(NBLK * LB // 1024):
                zg_inst = nc.gpsimd.dma_gather(
                    zg_all[:, k * 8:(k + 1) * 8, :], z_all[:],
                    zgi_sb[:, k * 64:(k + 1) * 64], 1024, 1024, DX,
                    queue_num=pool_q(), single_packet=False,
                )
                add_dep_helper(zg_inst.ins, ag_inst.ins,
                               reason="gather z after AllGather")

            # Wqk = Wq @ Wk^T (bf16); bqk = Wk @ bq  (after the z chain on PE)
            for a in range(4):
                pwt = qkps.tile([128, 512], f32, tag="qkp")
                for b in range(2):
                    nc.tensor.matmul(pwt[:, :DH], lhsT=wqT_sb[b][:, a * 128:(a + 1) * 128],
                                     rhs=wkT_sb[b][:], start=(b == 0), stop=(b == 1))
                nc.vector.tensor_copy(wqk_sb[a][:], pwt[:, :DH])
            for c in range(2):
                pb = qkps.tile([128, 512], f32, tag="qkp")
                for b in range(2):
                    nc.tensor.matmul(pb[:, :1], lhsT=wkT_sb[b][:, c * 128:(c + 1) * 128],
                                     rhs=bq_sb[:, b:b + 1],
                                     start=(b == 0), stop=(b == 1))
                nc.vector.tensor_copy(bqk_sb[c][:], pb[:, :1])

            # ---- phase QT/QK/LQ: transpose q, then qkT / logit_q ----
            qkT = [persist.tile([128, SLOT_PAD], bf16, tag=f"qkT{c}", name=f"qkT{c}") for c in range(2)]
            logit_q = persist.tile([N_TYP, SLOT_PAD], f32, tag="logit_q")
            with tc.tile_pool(name="qtm", bufs=2) as qtm:
                for sl in range(NCH):
                    qtmix = [None, None]
                    for half in range(2):
                        qm = qtm.tile([128, 4, 2, 128], bf16, tag=f"qTm{half}",
                                      name=f"qTm{half}")
                        qtmix[half] = qm
                        for t in range(4):
                            j = half * (SLOT_PAD // 128) + sl * 4 + t
                            pt = qtps.tile([128, 2, 128], bf16, tag="qtp")
                            for c in range(2):
                                tr = nc.tensor.transpose(pt[:, c, :],
                                                         qg[:, j, c * 128:(c + 1) * 128],
                                                         ident_bf[:])
                                if sl == 0 and half == 0 and t == 0 and c == 0:
                                    add_dep_helper(tr.ins, zmy_last.ins,
                                                   reason="keep z chain ahead of QT on PE")
                            qmc = nc.vector.tensor_copy(qm[:, t, :, :], pt[:])
                            if sl == 0 and half == 0 and t == 0:
                                add_dep_helper(qmc.ins, zrow_copy.ins,
                                               reason="keep z evac ahead of QT on DVE")

                    def qT_a(a):
                        return qtmix[a // 2][:, :, a % 2, :]

                    for c in range(2):
                        pq = qkps.tile([128, 512], f32, tag="qkp")
                        for a in range(4):
                            nc.tensor.matmul(pq[:], lhsT=wqk_sb[a][:, c * 128:(c + 1) * 128],
                                             rhs=qT_a(a), start=(a == 0), stop=(a == 3))
                        nc.scalar.activation(qkT[c][:, sl * 512:(sl + 1) * 512], pq[:],
                                             mybir.ActivationFunctionType.Identity,
                                             bias=bqk_sb[c][:, :1])
                    pl = lqps.tile([N_TYP, 512], f32, tag="lqp")
                    for a in range(4):
                        nc.tensor.matmul(pl[:], lhsT=wrel_sb[a][:], rhs=qT_a(a),
                                         start=(a == 0), stop=(a == 3))
                    nc.scalar.activation(logit_q[:, sl * 512:(sl + 1) * 512], pl[:],
                                         mybir.ActivationFunctionType.Identity,
                                         bias=brel_sb[:, :1])

            lqps_cm.__exit__(None, None, None)
            qkps_cm.__exit__(None, None, None)
            qtps_cm.__exit__(None, None, None)
            ztps_cm.__exit__(None, None, None)
            zps_cm.__exit__(None, None, None)

            # ---- phase S1: scores (with fused mask) / softmax / attnT ----
            aT = persist.tile([128, NBLK, NLT, CAP], bf16, tag="aT")
            hTb = [None, None]
            with (
                tc.tile_pool(name="sps", bufs=2, space="PSUM") as sps,
                tc.tile_pool(name="atps", bufs=3, space="PSUM") as atps,
            ):
                for b in range(NBLK):
                    if b % SB == 0:
                        for c in range(2):
                            hTb[c] = blk.tile([128, SB * LB], bf16, tag=f"hT{c}", name=f"hT{c}")
                            nc.scalar.dma_start(
                                hTb[c][:],
                                hwinT[c * 128:(c + 1) * 128, b * LB:(b + SB) * LB])
                    off = (b % SB) * LB

                    ps_s = sps.tile([CAP, LB], f32, tag="sps")
                    for c in range(2):
                        nc.tensor.matmul(ps_s[:], lhsT=qkT[c][:, b * CAP:b * CAP + CAP],
                                         rhs=hTb[c][:, off:off + LB],
                                         start=(c == 0), stop=False)
                    nc.tensor.matmul(ps_s[:], lhsT=onehot_sb[:, b * CAP:b * CAP + CAP],
                                     rhs=winmask_sb[:, b * LB:(b + 1) * LB],
                                     start=False, stop=True)
                    e = soft.tile([CAP, LB], bf16, tag="e", bufs=2)
                    den = soft.tile([CAP, 1], f32, tag="den")
                    nc.scalar.activation(e[:], ps_s[:], mybir.ActivationFunctionType.Exp,
                                         scale=float(SCALE), accum_out=den[:])
                    rec = soft.tile([CAP, 1], f32, tag="rec")
                    nc.vector.reciprocal(rec[:], den[:])
                    attn = soft.tile([CAP, LB], bf16, tag="attn", bufs=2)
                    nc.vector.tensor_scalar_mul(attn[:], e[:], rec[:])

                    pta = atps.tile([128, NLT, CAP], bf16, tag="atp")
                    for k in range(NLT):
                        nc.tensor.transpose(pta[:, k, :], attn[:, k * 128:(k + 1) * 128],
                                            ident_bf[:CAP, :CAP])
                    nc.vector.tensor_copy(aT[:, b, :, :], pta[:])

            # ---- phase S2: ctxT = zg^T @ aT ----
            ctxT = persist.tile([128, SLOT_PAD], bf16, tag="ctxT")
            if NBLK * CAP < SLOT_PAD:
                nc.vector.memset(ctxT[:, NBLK * CAP:], 0.0)
            with tc.tile_pool(name="cps", bufs=3, space="PSUM") as cps:
                for b in range(NBLK):
                    ps_c = cps.tile([DX, CAP], f32, tag="cps")
                    for k in range(NLT):
                        nc.tensor.matmul(ps_c[:], lhsT=zg_all[:, b * NLT + k, :],
                                         rhs=aT[:, b, k, :],
                                         start=(k == 0), stop=(k == NLT - 1))
                    nc.scalar.activation(ctxT[:, b * CAP:b * CAP + CAP], ps_c[:],
                                         mybir.ActivationFunctionType.Copy)

            # ---- phase L: logitT = logit_q + WrelC^T @ ctxT ----
            with tc.tile_pool(name="lps", bufs=2, space="PSUM") as lps:
                for ch in range(NCH):
                    pl = lps.tile([N_TYP, 512], f32, tag="lps")
                    nc.tensor.matmul(pl[:], lhsT=wrel_sb[4][:],
                                     rhs=ctxT[:, ch * 512:(ch + 1) * 512],
                                     start=True, stop=True)
                    lg = soft.tile([N_TYP, 512], f32, tag="lg", bufs=2)
                    nc.vector.tensor_add(lg[:], pl[:], logit_q[:, ch * 512:(ch + 1) * 512])
                    nc.sync.dma_start(logitT[:, ch * 512:(ch + 1) * 512], lg[:])

    nc.compile()
    return nc


def _wrap16(flat):
    """int16 gather-index layout: index i at [i % 16, i // 16], rows tiled to 128."""
    a = np.asarray(flat, np.int16).reshape(-1, 16).T
    return np.ascontiguousarray(np.tile(a, (8, 1)))


def _prep(mem, grp, pos2grp, h_grp, msk, idx, src, dst, typ, tok_emb, Wq, bq, Wk, bk, Wrel, brel):
    """Host-side sharding/layout. Integer index work + relayout only."""
    import ml_dtypes
    bf = ml_dtypes.bfloat16
    idx = np.asarray(idx, np.int64)
    src = np.asarray(src, np.int64)
    dst = np.asarray(dst, np.int64)
    mem = np.asarray(mem, np.int64)
    grp = np.asarray(grp, np.int64)
    pos2grp = np.asarray(pos2grp, np.int64)
    msk = np.asarray(msk)
    h_grp = np.asarray(h_grp, np.float32)
    tok_emb = np.asarray(tok_emb, np.float32)

    # ---- count matrix for segment_sum ----
    C = np.bincount(grp * N_TOK + mem, minlength=G * N_TOK).reshape(G, N_TOK).astype(np.float32)

    # ---- per-core windows ----
    starts = np.array([idx[d * MC] for d in range(NC)])
    ends = np.array([idx[(d + 1) * MC - 1] for d in range(NC)])
    BS = 8
    Wmax = int((ends - starts).max()) + 1
    W = -(-Wmax // (3 * BS)) * (3 * BS)

    maxc = 0
    for d in range(NC):
        blkid = (idx[d * MC:(d + 1) * MC] - starts[d]) // BS
        maxc = max(maxc, int(np.bincount(blkid).max()))
    if maxc > 128:
        BS = 4
        W = -(-Wmax // (3 * BS)) * (3 * BS)
        maxc = 0
        for d in range(NC):
            blkid = (idx[d * MC:(d + 1) * MC] - starts[d]) // BS
            maxc = max(maxc, int(np.bincount(blkid).max()))
        assert maxc <= 128, f"block occupancy {maxc} > 128 even at BS=4"
    CAP = -(-maxc // 32) * 32
    NBLK = W // BS
    SLOT_PAD = -(-(NBLK * CAP) // 1024) * 1024
    LB = BS * L

    tok_pad = np.vstack([tok_emb, np.zeros((NT_PAD - N_TOK, DX), np.float32)])
    # [128, KT, DX]: partition p holds rows t*128+p, contiguous per partition
    tok_bf = np.ascontiguousarray(
        tok_pad.astype(bf).reshape(KT, 128, DX).transpose(1, 0, 2))
    f8 = ml_dtypes.float8_e4m3
    wqT_h = np.ascontiguousarray(np.asarray(Wq, np.float32).T)
    wkT_h = np.ascontiguousarray(np.asarray(Wk, np.float32).T)
    bq_h = np.ascontiguousarray(np.asarray(bq, np.float32).reshape(2, 128).T)
    wrel_h = np.ascontiguousarray(np.asarray(Wrel, np.float32))
    brel_h = np.asarray(brel, np.float32).reshape(N_TYP, 1)

    h_flat = np.ascontiguousarray(h_grp.reshape(N_SEQ * L, DH))
    per_core = []
    slot_maps = []
    for d in range(NC):
        n_lo = int(starts[d])
        qid = idx[d * MC:(d + 1) * MC]
        qsrc = src[d * MC:(d + 1) * MC]
        qdst = dst[d * MC:(d + 1) * MC]

        hw = np.zeros((W * L, DH), np.float32)
        n_hi = min(n_lo + W, N_SEQ)
        hw[: (n_hi - n_lo) * L] = h_flat[n_lo * L: n_hi * L]
        hw_bf = hw.astype(bf)
        hwT_bf = np.ascontiguousarray(hw_bf.T)

        blkid = (qid - n_lo) // BS
        cnt = np.zeros(NBLK, np.int64)
        slot = np.zeros(MC, np.int64)
        for i in range(MC):
            b = blkid[i]
            slot[i] = b * CAP + cnt[b]
            cnt[b] += 1
        slot_maps.append(slot)

        qsi_h = np.zeros(SLOT_PAD, np.int64)
        qdi_h = np.zeros(SLOT_PAD, np.int64)
        qsi_h[slot] = (qid - n_lo) * L + qsrc
        qdi_h[slot] = (qid - n_lo) * L + qdst

        p2g_pad = np.zeros((W, L), np.int64)
        p2g_pad[: n_hi - n_lo] = pos2grp[n_lo:n_hi]

        # one-hot slot rows (which seq-in-block each slot belongs to)
        onehot_h = np.zeros((16, SLOT_PAD), np.float32)
        o = (qid - n_lo) % BS
        onehot_h[o, slot] = 1.0
        # per-column window masks: row j valid for cols of seq j-in-block
        winmask_h = np.full((16, W * L), 0.0, np.float32)
        wm = np.full((W, L), NEG, np.float32)
        valid = msk[n_lo:n_hi].astype(bool)
        wm[: n_hi - n_lo][valid] = 0.0
        wcol = np.arange(W) % BS
        for j in range(BS):
            rowj = np.full((W, L), NEG, np.float32)
            rowj[wcol == j] = wm[wcol == j]
            winmask_h[j] = rowj.reshape(-1)

        cm = np.vstack([C[d * GC:(d + 1) * GC].T,
                        np.zeros((NT_PAD - N_TOK, GC), np.float32)])
        per_core.append({
            "hwin": hw_bf, "hwinT": hwT_bf, "tokh": tok_bf,
            "cmat": np.ascontiguousarray(
                cm.astype(bf).reshape(KT4, 4, 128, GC).transpose(2, 0, 1, 3)),
            "wqT": wqT_h, "wkT": wkT_h, "bq": bq_h, "wrel": wrel_h, "brel": brel_h,
            "qgi": _wrap16(np.concatenate([qsi_h, qdi_h])),
            "zgi": _wrap16(p2g_pad.reshape(-1)),
            "onehot": onehot_h.astype(bf),
            "winmask": winmask_h.astype(bf),
        })
    return per_core, slot_maps, (W, NBLK, BS, CAP, SLOT_PAD)


def kernel(**inputs) -> np.ndarray:
    from concourse.bass_utils import run_bass_kernel_spmd

    per_core, slot_maps, key = _prep(**{k: inputs[k] for k in (
        "mem", "grp", "pos2grp", "h_grp", "msk", "idx", "src", "dst", "typ",
        "tok_emb", "Wq", "bq", "Wk", "bk", "Wrel", "brel")})
    if key not in _cache:
        _cache[key] = _build(*key)
    nc = _cache[key]
    res = run_bass_kernel_spmd(nc, per_core, core_ids=list(range(NC)))
    globals()["LAST_RESULT"] = res
    globals()["LAST_EXEC_NS"] = res.exec_time_ns
    out = np.empty((M, N_TYP), np.float32)
    for d in range(NC):
        out[d * MC:(d + 1) * MC] = res.results[d]["logitT"][:, slot_maps[d]].T
    return out


# revision 58
# speedup vs baseline: 1.1790x; 1.0897x over previous
"""Trainium2 Bass kernel for nn_AttentionModel (gnn_message_passing).

Distribution (8 cores):
  - Queries (M=8192) sharded into 8 contiguous chunks of 1024. idx is sorted,
    so each core's queries live in a contiguous window of sequences; the core
    receives h_grp (bf16) for just that window (row-major for gathers +
    transposed for matmuls).
  - segment_sum z: sharded by group. Each core computes z rows [512d, 512d+512)
    as a dense count-matrix matmul  z_d = C_d @ tok_emb  (both bf16; counts
    are tiny integers so C is exact), then AllGather (bf16, Shared output).
  - Attention is block-diagonal: queries of one sequence attend to its own 64
    positions. Blocks of BS=8 sequences; per-block query slots padded to a
    uniform CAP so the SPMD program is static.
  - q rows and z rows are fetched with batched dma_gather calls (<=1024
    indices each, the SWDGE descriptor-ring limit), spread across 4 SWDGE
    queues for parallel descriptor generation.
  - The (sequence-window + msk) mask is folded into the scores matmul as an
    extra K=16 rank-1 block (one-hot slot rows x per-column mask rows), so
    there is no amask tensor and no DVE mask add.
  - All compute-heavy matmuls run in bf16.
"""

import numpy as np

N_SEQ, L, DH, DX, M, G, N_TOK, N_MEM, N_TYP = 1024, 64, 256, 128, 8192, 4096, 10000, 262144, 64
NC = 8
MC = M // NC            # queries per core
GC = G // NC            # z-groups per core
NT_PAD = ((N_TOK + 511) // 512) * 512   # 10240
KT = NT_PAD // 128
KT4 = KT // 4           # 4-k-tile DMA batches
SCALE = 1.0 / np.sqrt(np.float32(DH))
NEG = -1.0e9
_SHARED_AG = True

_cache = {}


def _build(W, NBLK, BS, CAP, SLOT_PAD):
    import concourse.bacc as bacc
    import concourse.bass as bass
    import concourse.mybir as mybir
    import concourse.tile as tile
    from concourse.masks import make_identity
    from bass_rust import add_dep_helper

    f32 = mybir.dt.float32
    i16 = mybir.dt.int16
    bf16 = mybir.dt.bfloat16
    LB = BS * L                      # l-columns per block (512 for BS=8)
    NLT = LB // 128                  # l-chunks per block (4)
    NCH = SLOT_PAD // 512            # 512-slot chunks
    WL = W * L
    SB = 3                           # h superblock (NBLK % 3 == 0)

    f8 = mybir.dt.float8e4
    nc = bacc.Bacc("TRN2", target_bir_lowering=False, num_swdge_queues=4)

    hwin = nc.declare_dram_parameter("hwin", [WL, DH], bf16, isOutput=False)
    hwinT = nc.declare_dram_parameter("hwinT", [DH, WL], bf16, isOutput=False)
    tokh = nc.declare_dram_parameter("tokh", [128, KT, DX], bf16, isOutput=False)
    cmat = nc.declare_dram_parameter("cmat", [128, KT4, 4, GC], bf16, isOutput=False)
    wqT = nc.declare_dram_parameter("wqT", [DH, 2 * DH], f32, isOutput=False)
    wkT = nc.declare_dram_parameter("wkT", [DH, DH], f32, isOutput=False)
    bq = nc.declare_dram_parameter("bq", [128, 2], f32, isOutput=False)
    wrel = nc.declare_dram_parameter("wrel", [2 * DH + DX, N_TYP], f32, isOutput=False)
    brel = nc.declare_dram_parameter("brel", [N_TYP, 1], f32, isOutput=False)
    qgi = nc.declare_dram_parameter("qgi", [128, 2 * SLOT_PAD // 16], i16, isOutput=False)
    zgi = nc.declare_dram_parameter("zgi", [128, NBLK * LB // 16], i16, isOutput=False)
    onehot = nc.declare_dram_parameter("onehot", [16, SLOT_PAD], bf16, isOutput=False)
    winmask = nc.declare_dram_parameter("winmask", [16, WL], bf16, isOutput=False)
    logitT = nc.declare_dram_parameter("logitT", [N_TYP, SLOT_PAD], f32, isOutput=True)

    z_my = nc.dram_tensor("z_my", [GC, DX], bf16)
    z_all = nc.dram_tensor("z_all", [G, DX], bf16, addr_space="Shared" if _SHARED_AG else "Local")

    with tile.TileContext(nc) as tc:
        with (
            tc.tile_pool(name="const", bufs=1) as const,
            tc.tile_pool(name="persist", bufs=1) as persist,
            tc.tile_pool(name="zstream", bufs=2) as zstream,
            tc.tile_pool(name="blk", bufs=2) as blk,
            tc.tile_pool(name="soft", bufs=3) as soft,
        ):
            ident0 = const.tile([128, 128], f32)
            make_identity(nc, ident0[:])
            # DVE-homed identities: PE transposes then depend on one engine sem.
            ident = const.tile([128, 128], f32, tag="identW")
            nc.vector.tensor_copy(ident[:], ident0[:])
            ident_bf = const.tile([128, 128], bf16, tag="identB")
            nc.vector.tensor_copy(ident_bf[:], ident0[:])

            # ---- weights / small inputs ----
            # token table first: the z matmul chain needs it earliest
            th_all = persist.tile([128, KT, DX], bf16, tag="tokh")
            nc.scalar.dma_start(th_all[:, :KT // 2, :], tokh[:, :KT // 2, :])
            nc.scalar.dma_start(th_all[:, KT // 2:, :], tokh[:, KT // 2:, :])
            wqT_sb = [persist.tile([128, 2 * DH], f32, tag=f"wqT{j}", name=f"wqT{j}") for j in range(2)]
            for j in range(2):
                nc.scalar.dma_start(wqT_sb[j][:], wqT[j * 128:(j + 1) * 128, :])
            wkT_sb = [persist.tile([128, DH], f32, tag=f"wkT{j}", name=f"wkT{j}") for j in range(2)]
            for j in range(2):
                nc.scalar.dma_start(wkT_sb[j][:], wkT[j * 128:(j + 1) * 128, :])
            bq_sb = persist.tile([128, 2], f32, tag="bq")
            nc.scalar.dma_start(bq_sb[:], bq[:])
            wrel_sb = [persist.tile([128, N_TYP], bf16, tag=f"wrel{k}", name=f"wrel{k}") for k in range(5)]
            wrel_f32 = [persist.tile([128, N_TYP], f32, tag=f"wrelF{k}", name=f"wrelF{k}") for k in range(5)]
            for k in range(5):
                nc.scalar.dma_start(wrel_f32[k][:], wrel[k * 128:(k + 1) * 128, :])
                nc.vector.tensor_copy(wrel_sb[k][:], wrel_f32[k][:])
            brel_sb = persist.tile([N_TYP, 1], f32, tag="brel")
            nc.scalar.dma_start(brel_sb[:], brel[:])
            qgi_sb = persist.tile([128, 2 * SLOT_PAD // 16], i16, tag="qgi")
            nc.scalar.dma_start(qgi_sb[:], qgi[:])
            zgi_sb = persist.tile([128, NBLK * LB // 16], i16, tag="zgi")
            nc.scalar.dma_start(zgi_sb[:], zgi[:])
            onehot_sb = persist.tile([16, SLOT_PAD], bf16, tag="onehot")
            nc.scalar.dma_start(onehot_sb[:], onehot[:])
            winmask_sb = persist.tile([16, WL], bf16, tag="winmask")
            nc.scalar.dma_start(winmask_sb[:], winmask[:])

            wqk_sb = [persist.tile([128, DH], bf16, tag=f"wqk{a}", name=f"wqk{a}") for a in range(4)]
            bqk_sb = [persist.tile([128, 1], f32, tag=f"bqk{c}", name=f"bqk{c}") for c in range(2)]

            # front PSUM pools: 1 + 1 + 2 + 2 + 2 = 8 banks
            zps_cm = tc.tile_pool(name="zps", bufs=1, space="PSUM"); zps = zps_cm.__enter__()
            ztps_cm = tc.tile_pool(name="ztps", bufs=1, space="PSUM"); ztps = ztps_cm.__enter__()
            qtps_cm = tc.tile_pool(name="qtps", bufs=2, space="PSUM"); qtps = qtps_cm.__enter__()
            qkps_cm = tc.tile_pool(name="qkps", bufs=2, space="PSUM"); qkps = qkps_cm.__enter__()
            lqps_cm = tc.tile_pool(name="lqps", bufs=2, space="PSUM"); lqps = lqps_cm.__enter__()

            # ---- phase Z: z_d^T = tok^T @ C_d (bf16; counts shipped fp8,
            # upconverted on DVE/ACT), transpose, AllGather ----
            zdT = persist.tile([DX, GC], f32, tag="zdT")
            zrow = persist.tile([128, GC // 128 * DX], bf16, tag="zrow")
            zpsum = zps.tile([DX, GC], f32)
            # SWDGE sem lanes (8, round-robin over Pool DMA instrs) are locked
            # to one queue each: queue = lane %% 4 keeps the lane->queue map
            # consistent while every 4-instruction wave spans all 4 queues.
            pool_dma_n = [0]
            # lane -> queue table; lane 4 (the z_my write) is queue 0's only
            # occupant before the AllGather, so its completion isn't FIFO'd
            # behind slow gather drains.
            LANEQ = [1, 2, 3, 1, 0, 2, 3, 1]

            def pool_q():
                q = LANEQ[pool_dma_n[0] % 8]
                pool_dma_n[0] += 1
                return q

            # PE warmup: ~4us of dependency-free transposes so the HAM clock
            # gate opens (1.2 -> 2.4 GHz) before the z matmul chain starts.
            wrm = ztps.tile([128, GC // 128, 128], f32, tag="ztp")
            for i in range(36):
                nc.tensor.transpose(wrm[:, i % (GC // 128), :], ident[:], ident[:])

            c8_early = None
            for cb in range(KT4 // 2):      # 2 kb per HWDGE DMA chunk
                ck = zstream.tile([128, 2, 4, GC], bf16, tag="ck", bufs=4)
                c8_dma = nc.sync.dma_start(ck[:], cmat[:, 2 * cb:2 * cb + 2])
                if cb == 2:
                    c8_early = c8_dma
                for h in range(2):
                    kb = 2 * cb + h
                    for j in range(4):
                        nc.tensor.matmul(zpsum[:], lhsT=th_all[:, kb * 4 + j, :],
                                         rhs=ck[:, h, j, :],
                                         start=(kb == 0 and j == 0),
                                         stop=(kb == KT4 - 1 and j == 3))

            # ---- q gather: plain dma_gather (row-major), PE transposes ----
            # qg[p, j, :] = hwin[qidx[j*128+p], :]; j<16: src half, j>=16: dst.
            # SWDGE descriptor ring fits <=1024 idx per call. Deferred past the
            # z stream (dep on last cmat DMA) so the scheduler keeps all phase-Z
            # work ahead of gather-dependent PE instructions.
            qg = persist.tile([128, 2 * SLOT_PAD // 128, DH], bf16, tag="qg")
            for k in range(2 * SLOT_PAD // 1024):
                qg_inst = nc.gpsimd.dma_gather(
                    qg[:, k * 8:(k + 1) * 8, :], hwin[:],
                    qgi_sb[:, k * 64:(k + 1) * 64], 1024, 1024, DH,
                    queue_num=pool_q(), single_packet=False,
                )
                add_dep_helper(qg_inst.ins, c8_early.ins,
                               reason="stagger q gathers into the z stream")

            nc.vector.tensor_copy(zdT[:], zpsum[:])
            ptz = ztps.tile([128, GC // 128, 128], f32, tag="ztp")
            for c in range(GC // 128):
                nc.tensor.transpose(ptz[:, c, :], zdT[:, c * 128:(c + 1) * 128], ident[:])
            zrow_copy = nc.vector.tensor_copy(zrow[:], ptz[:])
            zmy_last = nc.gpsimd.dma_start(
                z_my.rearrange("(c p) x -> p c x", p=128), zrow[:])
            pool_q()  # account the SWDGE lane used by the z_my write
            ag_inst = nc.gpsimd.collective_compute(
                "AllGather", mybir.AluOpType.bypass,
                replica_groups=[list(range(NC))],
                ins=[z_my.ap().opt()], outs=[z_all.ap().opt()],
            )

            # ---- z gathers: dma_gather batches of 1024 rows (after AG) ----
            zg_all = persist.tile([128, NBLK * NLT, DX], bf16, tag="zg_all")
            for k in range(NBLK * LB // 1024):
                zg_inst = nc.gpsimd.dma_gather(
                    zg_all[:, k * 8:(k + 1) * 8, :], z_all[:],
                    zgi_sb[:, k * 64:(k + 1) * 64], 1024, 1024, DX,
                    queue_num=pool_q(), single_packet=False,
                )
                add_dep_helper(zg_inst.ins, ag_inst.ins,
                               reason="gather z after AllGather")

            # Wqk = Wq @ Wk^T (bf16); bqk = Wk @ bq  (after the z chain on PE)
            for a in range(4):
                pwt = qkps.tile([128, 512], f32, tag="qkp")
                for b in range(2):
                    nc.tensor.matmul(pwt[:, :DH], lhsT=wqT_sb[b][:, a * 128:(a + 1) * 128],
                                     rhs=wkT_sb[b][:], start=(b == 0), stop=(b == 1))
                nc.vector.tensor_copy(wqk_sb[a][:], pwt[:, :DH])
            for c in range(2):
                pb = qkps.tile([128, 512], f32, tag="qkp")
                for b in range(2):
                    nc.tensor.matmul(pb[:, :1], lhsT=wkT_sb[b][:, c * 128:(c + 1) * 128],
                                     rhs=bq_sb[:, b:b + 1],
                                     start=(b == 0), stop=(b == 1))
                nc.vector.tensor_copy(bqk_sb[c][:], pb[:, :1])

            # ---- phase QT/QK/LQ: transpose q, then qkT / logit_q ----
            qkT = [persist.tile([128, SLOT_PAD], bf16, tag=f"qkT{c}", name=f"qkT{c}") for c in range(2)]
            logit_q = persist.tile([N_TYP, SLOT_PAD], f32, tag="logit_q")
            with tc.tile_pool(name="qtm", bufs=2) as qtm:
                for sl in range(NCH):
                    qtmix = [None, None]
                    for half in range(2):
                        qm = qtm.tile([128, 4, 2, 128], bf16, tag=f"qTm{half}",
                                      name=f"qTm{half}")
                        qtmix[half] = qm
                        for t in range(4):
                            j = half * (SLOT_PAD // 128) + sl * 4 + t
                            pt = qtps.tile([128, 2, 128], bf16, tag="qtp")
                            for c in range(2):
                                tr = nc.tensor.transpose(pt[:, c, :],
                                                         qg[:, j, c * 128:(c + 1) * 128],
                                                         ident_bf[:])
                                if sl == 0 and half == 0 and t == 0 and c == 0:
                                    add_dep_helper(tr.ins, zmy_last.ins,
                                                   reason="keep z chain ahead of QT on PE")
                            qmc = nc.vector.tensor_copy(qm[:, t, :, :], pt[:])
                            if sl == 0 and half == 0 and t == 0:
                                add_dep_helper(qmc.ins, zrow_copy.ins,
                                               reason="keep z evac ahead of QT on DVE")

                    def qT_a(a):
                        return qtmix[a // 2][:, :, a % 2, :]

                    for c in range(2):
                        pq = qkps.tile([128, 512], f32, tag="qkp")
                        for a in range(4):
                            nc.tensor.matmul(pq[:], lhsT=wqk_sb[a][:, c * 128:(c + 1) * 128],
                                             rhs=qT_a(a), start=(a == 0), stop=(a == 3))
                        nc.scalar.activation(qkT[c][:, sl * 512:(sl + 1) * 512], pq[:],
                                             mybir.ActivationFunctionType.Identity,
                                             bias=bqk_sb[c][:, :1])
                    pl = lqps.tile([N_TYP, 512], f32, tag="lqp")
                    for a in range(4):
                        nc.tensor.matmul(pl[:], lhsT=wrel_sb[a][:], rhs=qT_a(a),
                                         start=(a == 0), stop=(a == 3))
                    nc.scalar.activation(logit_q[:, sl * 512:(sl + 1) * 512], pl[:],
                                         mybir.ActivationFunctionType.Identity,
                                         bias=brel_sb[:, :1])

            lqps_cm.__exit__(None, None, None)
            qkps_cm.__exit__(None, None, None)
            qtps_cm.__exit__(None, None, None)
            ztps_cm.__exit__(None, None, None)
            zps_cm.__exit__(None, None, None)

            # ---- phase S1: scores (with fused mask) / softmax / attnT ----
            aT = persist.tile([128, NBLK, NLT, CAP], bf16, tag="aT")
            hTb = [None, None]
            with (
                tc.tile_pool(name="sps", bufs=2, space="PSUM") as sps,
                tc.tile_pool(name="atps", bufs=3, space="PSUM") as atps,
            ):
                for b in range(NBLK):
                    if b % SB == 0:
                        for c in range(2):
                            hTb[c] = blk.tile([128, SB * LB], bf16, tag=f"hT{c}", name=f"hT{c}")
                            nc.scalar.dma_start(
                                hTb[c][:],
                                hwinT[c * 128:(c + 1) * 128, b * LB:(b + SB) * LB])
                    off = (b % SB) * LB

                    ps_s = sps.tile([CAP, LB], f32, tag="sps")
                    for c in range(2):
                        nc.tensor.matmul(ps_s[:], lhsT=qkT[c][:, b * CAP:b * CAP + CAP],
                                         rhs=hTb[c][:, off:off + LB],
                                         start=(c == 0), stop=False)
                    nc.tensor.matmul(ps_s[:], lhsT=onehot_sb[:, b * CAP:b * CAP + CAP],
                                     rhs=winmask_sb[:, b * LB:(b + 1) * LB],
                                     start=False, stop=True)
                    e = soft.tile([CAP, LB], bf16, tag="e", bufs=2)
                    den = soft.tile([CAP, 1], f32, tag="den")
                    nc.scalar.activation(e[:], ps_s[:], mybir.ActivationFunctionType.Exp,
                                         scale=float(SCALE), accum_out=den[:])
                    rec = soft.tile([CAP, 1], f32, tag="rec")
                    nc.vector.reciprocal(rec[:], den[:])
                    attn = soft.tile([CAP, LB], bf16, tag="attn", bufs=2)
                    nc.vector.tensor_scalar_mul(attn[:], e[:], rec[:])

                    pta = atps.tile([128, NLT, CAP], bf16, tag="atp")
                    for k in range(NLT):
                        nc.tensor.transpose(pta[:, k, :], attn[:, k * 128:(k + 1) * 128],
                                            ident_bf[:CAP, :CAP])
                    nc.vector.tensor_copy(aT[:, b, :, :], pta[:])

            # ---- phase S2: ctxT = zg^T @ aT ----
            ctxT = persist.tile([128, SLOT_PAD], bf16, tag="ctxT")
            if NBLK * CAP < SLOT_PAD:
                nc.vector.memset(ctxT[:, NBLK * CAP:], 0.0)
            with tc.tile_pool(name="cps", bufs=3, space="PSUM") as cps:
                for b in range(NBLK):
                    ps_c = cps.tile([DX, CAP], f32, tag="cps")
                    for k in range(NLT):
                        nc.tensor.matmul(ps_c[:], lhsT=zg_all[:, b * NLT + k, :],
                                         rhs=aT[:, b, k, :],
                                         start=(k == 0), stop=(k == NLT - 1))
                    nc.scalar.activation(ctxT[:, b * CAP:b * CAP + CAP], ps_c[:],
                                         mybir.ActivationFunctionType.Copy)

            # ---- phase L: logitT = logit_q + WrelC^T @ ctxT ----
            with tc.tile_pool(name="lps", bufs=2, space="PSUM") as lps:
                for ch in range(NCH):
                    pl = lps.tile([N_TYP, 512], f32, tag="lps")
                    nc.tensor.matmul(pl[:], lhsT=wrel_sb[4][:],
                                     rhs=ctxT[:, ch * 512:(ch + 1) * 512],
                                     start=True, stop=True)
                    lg = soft.tile([N_TYP, 512], f32, tag="lg", bufs=2)
                    nc.vector.tensor_add(lg[:], pl[:], logit_q[:, ch * 512:(ch + 1) * 512])
                    nc.sync.dma_start(logitT[:, ch * 512:(ch + 1) * 512], lg[:])

    nc.compile()
    return nc


def _wrap16(flat):
    """int16 gather-index layout: index i at [i % 16, i // 16], rows tiled to 128."""
    a = np.asarray(flat, np.int16).reshape(-1, 16).T
    return np.ascontiguousarray(np.tile(a, (8, 1)))


def _prep(mem, grp, pos2grp, h_grp, msk, idx, src, dst, typ, tok_emb, Wq, bq, Wk, bk, Wrel, brel):
    """Host-side sharding/layout. Integer index work + relayout only."""
    import ml_dtypes
    bf = ml_dtypes.bfloat16
    idx = np.asarray(idx, np.int64)
    src = np.asarray(src, np.int64)
    dst = np.asarray(dst, np.int64)
    mem = np.asarray(mem, np.int64)
    grp = np.asarray(grp, np.int64)
    pos2grp = np.asarray(pos2grp, np.int64)
    msk = np.asarray(msk)
    h_grp = np.asarray(h_grp, np.float32)
    tok_emb = np.asarray(tok_emb, np.float32)

    # ---- count matrix for segment_sum ----
    C = np.bincount(grp * N_TOK + mem, minlength=G * N_TOK).reshape(G, N_TOK).astype(np.float32)

    # ---- per-core windows ----
    starts = np.array([idx[d * MC] for d in range(NC)])
    ends = np.array([idx[(d + 1) * MC - 1] for d in range(NC)])
    BS = 8
    Wmax = int((ends - starts).max()) + 1
    W = -(-Wmax // (3 * BS)) * (3 * BS)

    maxc = 0
    for d in range(NC):
        blkid = (idx[d * MC:(d + 1) * MC] - starts[d]) // BS
        maxc = max(maxc, int(np.bincount(blkid).max()))
    if maxc > 128:
        BS = 4
        W = -(-Wmax // (3 * BS)) * (3 * BS)
        maxc = 0
        for d in range(NC):
            blkid = (idx[d * MC:(d + 1) * MC] - starts[d]) // BS
            maxc = max(maxc, int(np.bincount(blkid).max()))
        assert maxc <= 128, f"block occupancy {maxc} > 128 even at BS=4"
    CAP = -(-maxc // 32) * 32
    NBLK = W // BS
    SLOT_PAD = -(-(NBLK * CAP) // 1024) * 1024
    LB = BS * L

    tok_pad = np.vstack([tok_emb, np.zeros((NT_PAD - N_TOK, DX), np.float32)])
    # [128, KT, DX]: partition p holds rows t*128+p, contiguous per partition
    tok_bf = np.ascontiguousarray(
        tok_pad.astype(bf).reshape(KT, 128, DX).transpose(1, 0, 2))
    f8 = ml_dtypes.float8_e4m3
    wqT_h = np.ascontiguousarray(np.asarray(Wq, np.float32).T)
    wkT_h = np.ascontiguousarray(np.asarray(Wk, np.float32).T)
    bq_h = np.ascontiguousarray(np.asarray(bq, np.float32).reshape(2, 128).T)
    wrel_h = np.ascontiguousarray(np.asarray(Wrel, np.float32))
    brel_h = np.asarray(brel, np.float32).reshape(N_TYP, 1)

    h_flat = np.ascontiguousarray(h_grp.reshape(N_SEQ * L, DH))
    per_core = []
    slot_maps = []
    for d in range(NC):
        n_lo = int(starts[d])
        qid = idx[d * MC:(d + 1) * MC]
        qsrc = src[d * MC:(d + 1) * MC]
        qdst = dst[d * MC:(d + 1) * MC]

        hw = np.zeros((W * L, DH), np.float32)
        n_hi = min(n_lo + W, N_SEQ)
        hw[: (n_hi - n_lo) * L] = h_flat[n_lo * L: n_hi * L]
        hw_bf = hw.astype(bf)
        hwT_bf = np.ascontiguousarray(hw_bf.T)

        blkid = (qid - n_lo) // BS
        cnt = np.zeros(NBLK, np.int64)
        slot = np.zeros(MC, np.int64)
        for i in range(MC):
            b = blkid[i]
            slot[i] = b * CAP + cnt[b]
            cnt[b] += 1
        slot_maps.append(slot)

        qsi_h = np.zeros(SLOT_PAD, np.int64)
        qdi_h = np.zeros(SLOT_PAD, np.int64)
        qsi_h[slot] = (qid - n_lo) * L + qsrc
        qdi_h[slot] = (qid - n_lo) * L + qdst

        p2g_pad = np.zeros((W, L), np.int64)
        p2g_pad[: n_hi - n_lo] = pos2grp[n_lo:n_hi]

        # one-hot slot rows (which seq-in-block each slot belongs to)
        onehot_h = np.zeros((16, SLOT_PAD), np.float32)
        o = (qid - n_lo) % BS
        onehot_h[o, slot] = 1.0
        # per-column window masks: row j valid for cols of seq j-in-block
        winmask_h = np.full((16, W * L), 0.0, np.float32)
        wm = np.full((W, L), NEG, np.float32)
        valid = msk[n_lo:n_hi].astype(bool)
        wm[: n_hi - n_lo][valid] = 0.0
        wcol = np.arange(W) % BS
        for j in range(BS):
            rowj = np.full((W, L), NEG, np.float32)
            rowj[wcol == j] = wm[wcol == j]
            winmask_h[j] = rowj.reshape(-1)

        cm = np.vstack([C[d * GC:(d + 1) * GC].T,
                        np.zeros((NT_PAD - N_TOK, GC), np.float32)])
        per_core.append({
            "hwin": hw_bf, "hwinT": hwT_bf, "tokh": tok_bf,
            "cmat": np.ascontiguousarray(
                cm.astype(bf).reshape(KT4, 4, 128, GC).transpose(2, 0, 1, 3)),
            "wqT": wqT_h, "wkT": wkT_h, "bq": bq_h, "wrel": wrel_h, "brel": brel_h,
            "qgi": _wrap16(np.concatenate([qsi_h, qdi_h])),
            "zgi": _wrap16(p2g_pad.reshape(-1)),
            "onehot": onehot_h.astype(bf),
            "winmask": winmask_h.astype(bf),
        })
    return per_core, slot_maps, (W, NBLK, BS, CAP, SLOT_PAD)


def kernel(**inputs) -> np.ndarray:
    from concourse.bass_utils import run_bass_kernel_spmd

    per_core, slot_maps, key = _prep(**{k: inputs[k] for k in (
        "mem", "grp", "pos2grp", "h_grp", "msk", "idx", "src", "dst", "typ",
        "tok_emb", "Wq", "bq", "Wk", "bk", "Wrel", "brel")})
    if key not in _cache:
        _cache[key] = _build(*key)
    nc = _cache[key]
    res = run_bass_kernel_spmd(nc, per_core, core_ids=list(range(NC)))
    globals()["LAST_RESULT"] = res
    globals()["LAST_EXEC_NS"] = res.exec_time_ns
    out = np.empty((M, N_TYP), np.float32)
    for d in range(NC):
        out[d * MC:(d + 1) * MC] = res.results[d]["logitT"][:, slot_maps[d]].T
    return out
